# revision 27
# baseline (speedup 1.0000x reference)
"""Trainium2 Bass kernel for rank-1 attention + linear (nn_Attention).

Reference computation (S=256, B=128, D=4096):
    scores   = einsum('sbd,bd->bs', inp, hidden[0])      # dot each enc state with hidden
    attn     = softmax(scores, axis=1)                   # over S
    weighted = einsum('bs,sbd->bd', attn, inp)
    concat   = [weighted, hidden[0]]   # [B, 2D]
    out      = concat @ W.T + b        # [1, B, D]

Distribution over 8 NeuronCores: data-parallel over B for the attention
(16 batches per core); the linear's W is sharded over the output dim
(512 rows per core) with the weighted vectors exchanged by one AllGather.

All heavy tensors are fp16 on the host: halves HBM traffic, runs PE matmuls
at 1 cycle/row (f32 costs 4), and keeps enough mantissa for near-tie softmax
batches (bf16 would perturb attention weights ~30x more).

The batch loop is software-pipelined in two stages so no engine's in-order
instruction stream blocks on another engine's round trip:
  S1(b): one 2MB DMA for both s-tiles; hidden row replicated across the 128
         partitions (even batches: stride-0-source DMA, odd batches: gpsimd
         software broadcast - splits the load between SDMA and Q7); DVE
         tensor_tensor product (2x mode) whose free-dim sum is taken by the
         Scalar engine, plus a fused scalar_tensor_tensor for the other
         tile; gpsimd partition-max.
  S2(b): DVE max-fold; ACT exp (+row accum); gpsimd partition-sum into a
         per-batch column; UNNORMALIZED exp weights scattered into a
         column-masked matrix; 16 PE matmuls accumulate weighted sums in
         PSUM (groups of 8/4/4 batches at row bases 0/32/64).
The 1/sum normalization is folded into each group's PSUM evacuation (ACT
copy with a per-partition scale). W / hidden^T stream in per-batch slabs on
the SWDGE queue. The tail runs the hidden-half matmuls while the AllGather
exchanges the weighted vectors, then PE-transposes the gathered matrix and
finishes the remaining 32 matmuls + bias.
"""
import sys

if "/opt/trn_rl_repo" not in sys.path:
    sys.path.insert(0, "/opt/trn_rl_repo")

import numpy as np

MODE = "fp16"


# ----------------------------------------------------------------------------
# Program builder
# ----------------------------------------------------------------------------

def build_program(S=256, B=128, D=4096, n_cores=8, no_collective=False,
                  mode=MODE):
    """Build the SPMD Bass program. Returns finalized nc."""
    import concourse.bacc as bacc
    import concourse.bass_isa as bass_isa
    import concourse.mybir as mybir
    import concourse.tile as tile
    from concourse import library_config

    f32 = mybir.dt.float32
    dlo = mybir.dt.float16 if mode == "fp16" else f32
    tlo = dlo
    f8 = mybir.dt.float8e4

    P = 128
    Bc = B // n_cores                 # batches per core (16)
    Bh = Bc // 2                      # first exchange group size (8)
    Bq = Bc // 4                      # later exchange group sizes (4)
    ST = S // P                       # s-tiles per batch (2)
    F = 2 * D                         # concat feature dim (8192)
    DOUT = D // n_cores               # output-dim shard per core (512)
    NC_D = D // 512                   # 512-wide d-chunks for weighted MMs (8)
    NT_W = D // P                     # 128-wide transpose chunks (32)
    NKF = F // P                      # 128-wide k-chunks of the linear (64)

    # exchange groups: (first batch, count, wacc row base)
    GROUPS = [(0, Bh, 0), (Bh, Bq, 32), (Bh + Bq, Bq, 64)]

    def group_of(b):
        for gi, (b0, n, base) in enumerate(GROUPS):
            if b0 <= b < b0 + n:
                return gi, b - b0, n, base
        raise AssertionError

    nc = bacc.Bacc(None, target_bir_lowering=False)

    inp = nc.dram_tensor("inp", [S, Bc, D], dlo, kind="ExternalInput")
    hid = nc.dram_tensor("hid", [Bc, D], dlo, kind="ExternalInput")
    hidT3 = nc.dram_tensor("hidT3", [P, NT_W, B], dlo, kind="ExternalInput")
    wt3 = nc.dram_tensor("wt3", [P, NKF, DOUT], dlo, kind="ExternalInput")
    bias = nc.dram_tensor("bias", [1, DOUT], f32, kind="ExternalInput")
    ident = nc.dram_tensor("ident", [P, P], dlo, kind="ExternalInput")
    out = nc.dram_tensor("out", [B, DOUT], f32, kind="ExternalOutput")

    cc_in = nc.dram_tensor("cc_in", [Bc, D], dlo)
    cc_outA = nc.dram_tensor("cc_outA", [B // 2, D], dlo, addr_space="Shared")
    cc_outB = nc.dram_tensor("cc_outB", [B // 2, D], dlo, addr_space="Shared")

    with tile.TileContext(nc) as tc:
        import contextlib

        with contextlib.ExitStack() as ctx:
            persist = ctx.enter_context(tc.tile_pool(name="persist", bufs=1))

            nc.gpsimd.load_library(library_config.attn)

            # small prefetches on the otherwise-idle ACT queue
            ident_sb = persist.tile([P, P], tlo)
            nc.scalar.dma_start(out=ident_sb, in_=ident[:, :])
            bias_sb = persist.tile([1, DOUT], f32)
            bias_bc = persist.tile([B, DOUT], f32)

            # masked attn weights: col (b - group_start) of slice (t, b) is
            # batch b's attn column, everything else stays zero
            attn_diag = persist.tile([P, ST, Bc, Bh], tlo)
            nc.vector.memset(attn_diag[:, :, :, :], 0.0)

            wsump = ctx.enter_context(tc.tile_pool(name="wsump", bufs=1))
            wag = persist.tile([B, D], dlo)
            sigrow = [persist.tile([n, 1], f32, name=f"sigrow{g}")
                      for g, (_, n, _) in enumerate(GROUPS)]
            # col b holds batch b's exp-sum on every partition
            sigs_all = persist.tile([P, Bc], f32)

            # linear inputs streamed in slabs during the batch loop
            hT_sb = persist.tile([P, NT_W, B], tlo)
            wt_hi = persist.tile([P, NKF // 2, DOUT], tlo)
            wt_lo = persist.tile([P, NKF // 2, DOUT], tlo)

            def emit_prefetch(b):
                # 4 of 64 W k-chunks + 2 of 32 hidden^T chunks per batch,
                # on the SWDGE queue (cheap descriptor generation)
                tgt = wt_hi if b < Bh else wt_lo
                base = (NKF // 2) if b < Bh else 0
                c0 = 4 * (b % Bh)
                nc.gpsimd.dma_start(
                    out=tgt[:, c0 : c0 + 4, :],
                    in_=wt3[:, base + c0 : base + c0 + 4, :],
                )
                h0 = 2 * b
                nc.gpsimd.dma_start(
                    out=hT_sb[:, h0 : h0 + 2, :],
                    in_=hidT3[:, h0 : h0 + 2, :],
                )

            # ---------------- attention (skewed batch pipeline) -----------
            with contextlib.ExitStack() as loop_ctx:
                natp = loop_ctx.enter_context(tc.tile_pool(name="nat", bufs=3))
                hrowp = loop_ctx.enter_context(tc.tile_pool(name="hrow", bufs=1))
                hbp = loop_ctx.enter_context(tc.tile_pool(name="hb", bufs=3))
                prodAp = loop_ctx.enter_context(tc.tile_pool(name="prodA", bufs=1))
                prodBp = loop_ctx.enter_context(tc.tile_pool(name="prodB", bufs=3))
                smalls = loop_ctx.enter_context(tc.tile_pool(name="smalls", bufs=8))
                waccp = loop_ctx.enter_context(
                    tc.tile_pool(name="wacc", bufs=1, space="PSUM")
                )

                # per-group row bases 0/32/64 (matmul PSUM outputs must
                # start at partition 0/32/64)
                wacc = waccp.tile([68, D], f32)

                def emit_hb(b):
                    # replicate hid row b across all 128 partitions. Even
                    # batches: one DMA reading the host-replicated 8-row
                    # block 16x (64KB contiguous reads). Odd batches: gpsimd
                    # software broadcast (Q7 cycles, no SDMA bandwidth).
                    hb = hbp.tile([P, D], dlo, tag="hb")
                    if b % 2 == 1:
                        hrow = hrowp.tile([1, D], dlo, tag="hrow")
                        nc.gpsimd.dma_start(out=hrow, in_=hid[b : b + 1, :])
                        nc.gpsimd.partition_broadcast(hb, hrow)
                    else:
                        nc.gpsimd.dma_start(
                            out=hb,
                            in_=hid[b : b + 1, :].unsqueeze(1).to_broadcast(
                                [1, P, D]
                            ),
                        )
                    return hb

                junk = persist.tile([P, D], f8)

                hbs = {0: emit_hb(0), 1: emit_hb(1)}
                st = {}          # per-batch live tiles across stages

                def stage1(b):
                    if b + 2 < Bc:
                        hbs[b + 2] = emit_hb(b + 2)
                    emit_prefetch(b)
                    hb = hbs.pop(b)
                    nat2 = natp.tile([P, ST, D], tlo, tag="nat")
                    nc.sync.dma_start(
                        out=nat2,
                        in_=inp[:, b, :].rearrange("(t p) d -> p t d", p=P),
                    )
                    sc_b = smalls.tile([P, ST], f32, tag="sc")
                    prodB = prodBp.tile([P, D], dlo, tag="prodB")
                    nc.vector.tensor_tensor(
                        out=prodB, in0=nat2[:, 1, :], in1=hb[:, :],
                        op=mybir.AluOpType.mult,
                    )
                    nc.scalar.activation(
                        out=junk, in_=prodB,
                        func=mybir.ActivationFunctionType.Copy,
                        bias=0.0, scale=1.0,
                        accum_out=sc_b[:, 1:2],
                    )
                    prodA = prodAp.tile([P, D], f8, tag="prodA")
                    nc.vector.scalar_tensor_tensor(
                        out=prodA,
                        in0=nat2[:, 0, :],
                        scalar=1.0,
                        in1=hb[:, :],
                        op0=mybir.AluOpType.mult,
                        op1=mybir.AluOpType.mult,
                        accum_out=sc_b[:, 0:1],
                    )
                    mx2 = smalls.tile([P, ST], f32, tag="mx2")
                    nc.gpsimd.partition_all_reduce(
                        mx2, sc_b, channels=P, reduce_op=bass_isa.ReduceOp.max
                    )
                    st[b] = {"nat2": nat2, "sc": sc_b, "mx2": mx2}

                def stage2(b):
                    t_ = st.pop(b)
                    gi, bb, gn, base = group_of(b)
                    negm = smalls.tile([P, 1], f32, tag="negm")
                    nc.vector.tensor_reduce(
                        out=negm, in_=t_["mx2"], axis=mybir.AxisListType.X,
                        op=mybir.AluOpType.max, negate=True,
                    )
                    e_b = smalls.tile([P, ST], f32, tag="e_b")
                    s1 = smalls.tile([P, 1], f32, tag="s1")
                    nc.scalar.activation(
                        out=e_b, in_=t_["sc"],
                        func=mybir.ActivationFunctionType.Exp,
                        bias=negm, scale=1.0, accum_out=s1,
                    )
                    # batch b's exp-sum lands in column b (all partitions)
                    nc.gpsimd.partition_all_reduce(
                        sigs_all[:, b : b + 1], s1, channels=P,
                        reduce_op=bass_isa.ReduceOp.add,
                    )
                    # scatter UNNORMALIZED weights; 1/sum is applied to the
                    # accumulated rows at evacuation time
                    nc.scalar.activation(
                        out=attn_diag[:, :, b, bb : bb + 1],
                        in_=e_b.unsqueeze(2),
                        func=mybir.ActivationFunctionType.Copy,
                    )
                    for t in range(ST):
                        for c in range(NC_D):
                            nc.tensor.matmul(
                                wacc[base : base + gn, c * 512 : (c + 1) * 512],
                                attn_diag[:, t, b, 0:gn],
                                t_["nat2"][:, t, c * 512 : (c + 1) * 512],
                                start=(bb == 0 and t == 0),
                                stop=(bb == gn - 1 and t == ST - 1),
                            )
                    if bb == gn - 1:
                        # group complete: normalize while evacuating, then
                        # stage into the exchange buffer
                        b0 = GROUPS[gi][0]
                        for q in range(gn):
                            nc.gpsimd.dma_start(
                                out=sigrow[gi][q : q + 1, :],
                                in_=sigs_all[0:1, b0 + q : b0 + q + 1],
                            )
                        rrow = smalls.tile([gn, 1], f32, tag=f"rrow{gi}",
                                           name=f"rrow{gi}")
                        nc.vector.reciprocal(rrow, sigrow[gi])
                        wsum = wsump.tile([Bh, D], dlo, tag="wsum")
                        nc.scalar.activation(
                            out=wsum[0:gn, :], in_=wacc[base : base + gn, :],
                            func=mybir.ActivationFunctionType.Copy,
                            scale=rrow,
                        )
                        nc.gpsimd.dma_start(
                            out=cc_in[b0 : b0 + gn, :], in_=wsum[0:gn, :]
                        )
                        if gi == 0:
                            # first half exchanged while batches 8-15 run
                            if no_collective:
                                for k in range(n_cores):
                                    nc.gpsimd.dma_start(
                                        out=cc_outA[k * Bh : (k + 1) * Bh, :],
                                        in_=cc_in[0:Bh, :],
                                    )
                            else:
                                nc.gpsimd.collective_compute(
                                    "AllGather",
                                    mybir.AluOpType.bypass,
                                    replica_groups=[list(range(n_cores))],
                                    ins=[cc_in[0:Bh, :]],
                                    outs=[cc_outA[:, :]],
                                )
                            for k in range(n_cores):
                                nc.gpsimd.dma_start(
                                    out=wag[k * Bc : k * Bc + Bh, :],
                                    in_=cc_outA[k * Bh : (k + 1) * Bh, :],
                                )

                for i in range(Bc + 1):
                    if i < Bc:
                        stage1(i)
                    if 0 <= i - 1 < Bc:
                        stage2(i - 1)

                # bias prefetch for the tail (ACT/Pool idle by now)
                nc.scalar.dma_start(out=bias_sb, in_=bias[:, :])
                nc.gpsimd.partition_broadcast(bias_bc, bias_sb)

            # ---------------- linear ----------------
            with contextlib.ExitStack() as lin_ctx:
                tailp = lin_ctx.enter_context(tc.tile_pool(name="tail", bufs=1))
                wTp = lin_ctx.enter_context(tc.tile_pool(name="wTp", bufs=NT_W))
                tpp = lin_ctx.enter_context(
                    tc.tile_pool(name="tp", bufs=4, space="PSUM")
                )
                linp = lin_ctx.enter_context(
                    tc.tile_pool(name="lin", bufs=1, space="PSUM")
                )

                out_ps = linp.tile([B, DOUT], f32)

                # hidden half first: lhsT chunks from the prefetched hidT.
                # Emitted BEFORE the collective so its DMA sem waits don't
                # transitively include the exchange.
                for c in range(NKF // 2):
                    nc.tensor.matmul(
                        out_ps,
                        hT_sb[:, c, :],
                        wt_hi[:, c, :],
                        start=(c == 0),
                        stop=False,
                    )

                # second-half exchange (first half ran mid-loop)
                if no_collective:
                    for k in range(n_cores):
                        nc.gpsimd.dma_start(
                            out=cc_outB[k * Bh : (k + 1) * Bh, :],
                            in_=cc_in[Bh:Bc, :],
                        )
                else:
                    nc.gpsimd.collective_compute(
                        "AllGather",
                        mybir.AluOpType.bypass,
                        replica_groups=[list(range(n_cores))],
                        ins=[cc_in[Bh:Bc, :]],
                        outs=[cc_outB[:, :]],
                    )
                for k in range(n_cores):
                    nc.gpsimd.dma_start(
                        out=wag[k * Bc + Bh : (k + 1) * Bc, :],
                        in_=cc_outB[k * Bh : (k + 1) * Bh, :],
                    )

                # weighted half: transpose the gathered matrix on PE
                wTs = []
                for c in range(NT_W):
                    tp_ps = tpp.tile([P, B], tlo, tag="tp")
                    nc.tensor.transpose(
                        tp_ps, wag[:, c * P : (c + 1) * P], ident_sb[:B, :B]
                    )
                    wT = wTp.tile([P, B], tlo, tag="wT")
                    nc.vector.tensor_copy(wT, tp_ps)
                    wTs.append(wT)

                for c in range(NT_W):
                    nc.tensor.matmul(
                        out_ps,
                        wTs[c],
                        wt_lo[:, c, :],
                        start=False,
                        stop=(c == NT_W - 1),
                    )

                # bias add + store
                out_sb = tailp.tile([B, DOUT], f32)
                nc.vector.tensor_add(out_sb, out_ps, bias_bc)
                nc.sync.dma_start(out=out[:, :], in_=out_sb)

    nc.finalize()
    return nc


_CACHE = {}


def _get_program(S, B, D, n_cores):
    key = (S, B, D, n_cores)
    if key not in _CACHE:
        _CACHE[key] = build_program(S, B, D, n_cores)
    return _CACHE[key]


def make_in_maps(inp, hidden, W, b, n_cores=8):
    """Shard host inputs into per-core input maps."""
    S, B, D = inp.shape
    Bc = B // n_cores
    DOUT = W.shape[0] // n_cores
    P = 128
    F = 2 * D
    NKF = F // P
    NT = D // P
    lo = np.float16 if MODE == "fp16" else np.float32
    inp = np.asarray(inp, dtype=np.float32).astype(lo)
    hid0 = np.asarray(hidden[0], dtype=np.float32).astype(lo)   # [B, D]
    # hidden^T in partition-major chunk layout [P, NT, B]
    hidT = np.ascontiguousarray(hid0.T)                         # [D, B]
    hidT3 = np.ascontiguousarray(
        hidT.reshape(NT, P, B).transpose(1, 0, 2))
    wtT = np.asarray(W, dtype=np.float32).T.astype(lo)          # [F, 4096]
    ident = np.eye(128, dtype=np.float32).astype(lo)
    in_maps = []
    for k in range(n_cores):
        wtk = wtT[:, k * DOUT : (k + 1) * DOUT]                 # [F, DOUT]
        wt3 = np.ascontiguousarray(
            wtk.reshape(NKF, P, DOUT).transpose(1, 0, 2))       # [P, NKF, DOUT]
        in_maps.append(
            {
                "inp": np.ascontiguousarray(inp[:, k * Bc : (k + 1) * Bc, :]),
                "hid": np.ascontiguousarray(hid0[k * Bc : (k + 1) * Bc, :]),
                "hidT3": hidT3,
                "wt3": wt3,
                "bias": np.ascontiguousarray(
                    np.asarray(b[k * DOUT : (k + 1) * DOUT], dtype=np.float32)
                    .reshape(1, DOUT)
                ),
                "ident": ident,
            }
        )
    return in_maps


def kernel(inp, hidden, W, b, trace=False):
    from concourse.bass_utils import run_bass_kernel_spmd

    inp = np.asarray(inp, dtype=np.float32)
    hidden = np.asarray(hidden, dtype=np.float32)
    W = np.asarray(W, dtype=np.float32)
    b = np.asarray(b, dtype=np.float32)

    S, B, D = inp.shape
    n_cores = 8
    nc = _get_program(S, B, D, n_cores)
    in_maps = make_in_maps(inp, hidden, W, b, n_cores)
    res = run_bass_kernel_spmd(nc, in_maps, core_ids=list(range(n_cores)))
    outs = [res.results[k]["out"] for k in range(n_cores)]
    full = np.concatenate(outs, axis=1)  # [B, D]
    if trace:
        return full[None, :, :], res
    return full[None, :, :]


# revision 28
# speedup vs baseline: 1.1027x; 1.1027x over previous
"""Trainium2 Bass kernel for rank-1 attention + linear (nn_Attention).

Reference computation (S=256, B=128, D=4096):
    scores   = einsum('sbd,bd->bs', inp, hidden[0])      # dot each enc state with hidden
    attn     = softmax(scores, axis=1)                   # over S
    weighted = einsum('bs,sbd->bd', attn, inp)
    concat   = [weighted, hidden[0]]   # [B, 2D]
    out      = concat @ W.T + b        # [1, B, D]

Distribution over 8 NeuronCores: data-parallel over B for the attention
(16 batches per core); the linear's W is sharded over the output dim
(512 rows per core) with the weighted vectors exchanged by one AllGather.

All heavy tensors are fp16 on the host: halves HBM traffic, runs PE matmuls
at 1 cycle/row (f32 costs 4), and keeps enough mantissa for near-tie softmax
batches (bf16 would perturb attention weights ~30x more).

The batch loop is software-pipelined in two stages so no engine's in-order
instruction stream blocks on another engine's round trip:
  S1(b): one 2MB DMA for both s-tiles; hidden row replicated across the 128
         partitions (even batches: stride-0-source DMA, odd batches: gpsimd
         software broadcast - splits the load between SDMA and Q7); DVE
         tensor_tensor product (2x mode) whose free-dim sum is taken by the
         Scalar engine, plus a fused scalar_tensor_tensor for the other
         tile; gpsimd partition-max.
  S2(b): DVE max-fold; ACT exp (+row accum); gpsimd partition-sum into a
         per-batch column; UNNORMALIZED exp weights scattered into a
         column-masked matrix; 16 PE matmuls accumulate weighted sums in
         PSUM (groups of 8/4/4 batches at row bases 0/32/64).
The 1/sum normalization is folded into each group's PSUM evacuation (ACT
copy with a per-partition scale). W / hidden^T stream in per-batch slabs on
the SWDGE queue. The tail runs the hidden-half matmuls while the AllGather
exchanges the weighted vectors, then PE-transposes the gathered matrix and
finishes the remaining 32 matmuls + bias.
"""
import sys

if "/opt/trn_rl_repo" not in sys.path:
    sys.path.insert(0, "/opt/trn_rl_repo")

import numpy as np

MODE = "fp16"


# ----------------------------------------------------------------------------
# Program builder
# ----------------------------------------------------------------------------

def build_program(S=256, B=128, D=4096, n_cores=8, no_collective=False,
                  mode=MODE):
    """Build the SPMD Bass program. Returns finalized nc."""
    import concourse.bacc as bacc
    import concourse.bass_isa as bass_isa
    import concourse.mybir as mybir
    import concourse.tile as tile
    from concourse import library_config

    f32 = mybir.dt.float32
    dlo = mybir.dt.float16 if mode == "fp16" else f32
    tlo = dlo
    f8 = mybir.dt.float8e4

    P = 128
    Bc = B // n_cores                 # batches per core (16)
    Bh = Bc // 2                      # first exchange group size (8)
    Bq = Bc // 4                      # later exchange group sizes (4)
    ST = S // P                       # s-tiles per batch (2)
    F = 2 * D                         # concat feature dim (8192)
    DOUT = D // n_cores               # output-dim shard per core (512)
    NC_D = D // 512                   # 512-wide d-chunks for weighted MMs (8)
    NT_W = D // P                     # 128-wide transpose chunks (32)
    NKF = F // P                      # 128-wide k-chunks of the linear (64)

    # exchange groups: (first batch, count, wacc row base)
    GROUPS = [(0, Bh, 0), (Bh, Bq, 32), (Bh + Bq, Bq, 64)]

    def group_of(b):
        for gi, (b0, n, base) in enumerate(GROUPS):
            if b0 <= b < b0 + n:
                return gi, b - b0, n, base
        raise AssertionError

    nc = bacc.Bacc(None, target_bir_lowering=False)

    inp = nc.dram_tensor("inp", [S, Bc, D], dlo, kind="ExternalInput")
    hid = nc.dram_tensor("hid", [Bc, D], dlo, kind="ExternalInput")
    hidT3 = nc.dram_tensor("hidT3", [P, NT_W, B], dlo, kind="ExternalInput")
    wt3 = nc.dram_tensor("wt3", [P, NKF, DOUT], dlo, kind="ExternalInput")
    bias = nc.dram_tensor("bias", [1, DOUT], f32, kind="ExternalInput")
    ident = nc.dram_tensor("ident", [P, P], dlo, kind="ExternalInput")
    out = nc.dram_tensor("out", [B, DOUT], f32, kind="ExternalOutput")

    cc_in = nc.dram_tensor("cc_in", [Bc, D], dlo)
    cc_outA = nc.dram_tensor("cc_outA", [B // 2, D], dlo, addr_space="Shared")
    cc_outB = nc.dram_tensor("cc_outB", [B // 2, D], dlo, addr_space="Shared")

    with tile.TileContext(nc) as tc:
        import contextlib

        with contextlib.ExitStack() as ctx:
            persist = ctx.enter_context(tc.tile_pool(name="persist", bufs=1))

            nc.gpsimd.load_library(library_config.attn)

            # small prefetches on the otherwise-idle ACT queue
            ident_sb = persist.tile([P, P], tlo)
            nc.scalar.dma_start(out=ident_sb, in_=ident[:, :])
            bias_sb = persist.tile([1, DOUT], f32)
            bias_bc = persist.tile([B, DOUT], f32)

            # masked attn weights: col (b - group_start) of slice (t, b) is
            # batch b's attn column, everything else stays zero
            attn_diag = persist.tile([P, ST, Bc, Bh], tlo)
            nc.vector.memset(attn_diag[:, :, :, :], 0.0)

            wsump = ctx.enter_context(tc.tile_pool(name="wsump", bufs=1))
            wag = persist.tile([B, D], dlo)
            sigrow = [persist.tile([n, 1], f32, name=f"sigrow{g}")
                      for g, (_, n, _) in enumerate(GROUPS)]
            # col b holds batch b's exp-sum on every partition
            sigs_all = persist.tile([P, Bc], f32)

            # linear inputs streamed in slabs during the batch loop
            hT_sb = persist.tile([P, NT_W, B], tlo)
            wt_hi = persist.tile([P, NKF // 2, DOUT], tlo)
            wt_lo = persist.tile([P, NKF // 2, DOUT], tlo)

            def emit_prefetch(b):
                # 4 of 64 W k-chunks + 2 of 32 hidden^T chunks per batch,
                # on the SWDGE queue (cheap descriptor generation)
                tgt = wt_hi if b < Bh else wt_lo
                base = (NKF // 2) if b < Bh else 0
                c0 = 4 * (b % Bh)
                nc.gpsimd.dma_start(
                    out=tgt[:, c0 : c0 + 4, :],
                    in_=wt3[:, base + c0 : base + c0 + 4, :],
                )
                h0 = 2 * b
                nc.gpsimd.dma_start(
                    out=hT_sb[:, h0 : h0 + 2, :],
                    in_=hidT3[:, h0 : h0 + 2, :],
                )

            # ---------------- attention (skewed batch pipeline) -----------
            with contextlib.ExitStack() as loop_ctx:
                natp = loop_ctx.enter_context(tc.tile_pool(name="nat", bufs=3))
                hrowp = loop_ctx.enter_context(tc.tile_pool(name="hrow", bufs=1))
                hbp = loop_ctx.enter_context(tc.tile_pool(name="hb", bufs=3))
                prodAp = loop_ctx.enter_context(tc.tile_pool(name="prodA", bufs=1))
                prodBp = loop_ctx.enter_context(tc.tile_pool(name="prodB", bufs=2))
                smalls = loop_ctx.enter_context(tc.tile_pool(name="smalls", bufs=6))
                waccp = loop_ctx.enter_context(
                    tc.tile_pool(name="wacc", bufs=1, space="PSUM")
                )

                # per-group row bases 0/32/64 (matmul PSUM outputs must
                # start at partition 0/32/64)
                wacc = waccp.tile([68, D], f32)

                def emit_hb(b):
                    # replicate hid row b across all 128 partitions. Even
                    # batches: one DMA reading the host-replicated 8-row
                    # block 16x (64KB contiguous reads). Odd batches: gpsimd
                    # software broadcast (Q7 cycles, no SDMA bandwidth).
                    hb = hbp.tile([P, D], dlo, tag="hb")
                    if b % 2 == 1:
                        hrow = hrowp.tile([1, D], dlo, tag="hrow")
                        nc.gpsimd.dma_start(out=hrow, in_=hid[b : b + 1, :])
                        nc.gpsimd.partition_broadcast(hb, hrow)
                    else:
                        nc.gpsimd.dma_start(
                            out=hb,
                            in_=hid[b : b + 1, :].unsqueeze(1).to_broadcast(
                                [1, P, D]
                            ),
                        )
                    return hb

                junk = persist.tile([P, D], f8)

                hbs = {0: emit_hb(0), 1: emit_hb(1)}
                st = {}          # per-batch live tiles across stages

                def stage1(b):
                    if b + 2 < Bc:
                        hbs[b + 2] = emit_hb(b + 2)
                    emit_prefetch(b)
                    hb = hbs.pop(b)
                    nat2 = natp.tile([P, ST, D], tlo, tag="nat")
                    nc.sync.dma_start(
                        out=nat2,
                        in_=inp[:, b, :].rearrange("(t p) d -> p t d", p=P),
                    )
                    sc_b = smalls.tile([P, ST], f32, tag="sc")
                    prodB = prodBp.tile([P, D], dlo, tag="prodB")
                    nc.vector.tensor_tensor(
                        out=prodB, in0=nat2[:, 1, :], in1=hb[:, :],
                        op=mybir.AluOpType.mult,
                    )
                    nc.scalar.activation(
                        out=junk, in_=prodB,
                        func=mybir.ActivationFunctionType.Copy,
                        bias=0.0, scale=1.0,
                        accum_out=sc_b[:, 1:2],
                    )
                    prodA = prodAp.tile([P, D], f8, tag="prodA")
                    nc.vector.scalar_tensor_tensor(
                        out=prodA,
                        in0=nat2[:, 0, :],
                        scalar=1.0,
                        in1=hb[:, :],
                        op0=mybir.AluOpType.mult,
                        op1=mybir.AluOpType.mult,
                        accum_out=sc_b[:, 0:1],
                    )
                    mx2 = smalls.tile([P, ST], f32, tag="mx2")
                    nc.gpsimd.partition_all_reduce(
                        mx2, sc_b, channels=P, reduce_op=bass_isa.ReduceOp.max
                    )
                    st[b] = {"nat2": nat2, "sc": sc_b, "mx2": mx2}

                def stage2(b):
                    t_ = st.pop(b)
                    gi, bb, gn, base = group_of(b)
                    negm = smalls.tile([P, 1], f32, tag="negm")
                    nc.vector.tensor_reduce(
                        out=negm, in_=t_["mx2"], axis=mybir.AxisListType.X,
                        op=mybir.AluOpType.max, negate=True,
                    )
                    e_b = smalls.tile([P, ST], f32, tag="e_b")
                    s1 = smalls.tile([P, 1], f32, tag="s1")
                    nc.scalar.activation(
                        out=e_b, in_=t_["sc"],
                        func=mybir.ActivationFunctionType.Exp,
                        bias=negm, scale=1.0, accum_out=s1,
                    )
                    # batch b's exp-sum lands in column b (all partitions)
                    nc.gpsimd.partition_all_reduce(
                        sigs_all[:, b : b + 1], s1, channels=P,
                        reduce_op=bass_isa.ReduceOp.add,
                    )
                    # scatter UNNORMALIZED weights; 1/sum is applied to the
                    # accumulated rows at evacuation time
                    nc.scalar.activation(
                        out=attn_diag[:, :, b, bb : bb + 1],
                        in_=e_b.unsqueeze(2),
                        func=mybir.ActivationFunctionType.Copy,
                    )
                    for t in range(ST):
                        for c in range(NC_D):
                            nc.tensor.matmul(
                                wacc[base : base + gn, c * 512 : (c + 1) * 512],
                                attn_diag[:, t, b, 0:gn],
                                t_["nat2"][:, t, c * 512 : (c + 1) * 512],
                                start=(bb == 0 and t == 0),
                                stop=(bb == gn - 1 and t == ST - 1),
                            )
                    if bb == gn - 1:
                        # group complete: normalize while evacuating, then
                        # stage into the exchange buffer
                        b0 = GROUPS[gi][0]
                        for q in range(gn):
                            nc.gpsimd.dma_start(
                                out=sigrow[gi][q : q + 1, :],
                                in_=sigs_all[0:1, b0 + q : b0 + q + 1],
                            )
                        rrow = smalls.tile([gn, 1], f32, tag=f"rrow{gi}",
                                           name=f"rrow{gi}")
                        nc.vector.reciprocal(rrow, sigrow[gi])
                        wsum = wsump.tile([Bh, D], dlo, tag="wsum")
                        nc.scalar.activation(
                            out=wsum[0:gn, :], in_=wacc[base : base + gn, :],
                            func=mybir.ActivationFunctionType.Copy,
                            scale=rrow,
                        )
                        nc.gpsimd.dma_start(
                            out=cc_in[b0 : b0 + gn, :], in_=wsum[0:gn, :]
                        )
                        if gi == 0:
                            # first half exchanged while batches 8-15 run
                            if no_collective:
                                for k in range(n_cores):
                                    nc.gpsimd.dma_start(
                                        out=cc_outA[k * Bh : (k + 1) * Bh, :],
                                        in_=cc_in[0:Bh, :],
                                    )
                            else:
                                nc.gpsimd.collective_compute(
                                    "AllGather",
                                    mybir.AluOpType.bypass,
                                    replica_groups=[list(range(n_cores))],
                                    ins=[cc_in[0:Bh, :]],
                                    outs=[cc_outA[:, :]],
                                )
                            for k in range(n_cores):
                                nc.gpsimd.dma_start(
                                    out=wag[k * Bc : k * Bc + Bh, :],
                                    in_=cc_outA[k * Bh : (k + 1) * Bh, :],
                                )

                for i in range(Bc + 1):
                    if i < Bc:
                        stage1(i)
                    if 0 <= i - 1 < Bc:
                        stage2(i - 1)

                # bias prefetch for the tail (ACT/Pool idle by now)
                nc.scalar.dma_start(out=bias_sb, in_=bias[:, :])
                nc.gpsimd.partition_broadcast(bias_bc, bias_sb)

            # ---------------- linear ----------------
            with contextlib.ExitStack() as lin_ctx:
                tailp = lin_ctx.enter_context(tc.tile_pool(name="tail", bufs=1))
                wTp = lin_ctx.enter_context(tc.tile_pool(name="wTp", bufs=NT_W))
                tpp = lin_ctx.enter_context(
                    tc.tile_pool(name="tp", bufs=4, space="PSUM")
                )
                linp = lin_ctx.enter_context(
                    tc.tile_pool(name="lin", bufs=1, space="PSUM")
                )

                out_ps = linp.tile([B, DOUT], f32)

                # hidden half first: lhsT chunks from the prefetched hidT.
                # Emitted BEFORE the collective so its DMA sem waits don't
                # transitively include the exchange.
                for c in range(NKF // 2):
                    nc.tensor.matmul(
                        out_ps,
                        hT_sb[:, c, :],
                        wt_hi[:, c, :],
                        start=(c == 0),
                        stop=False,
                    )

                # second-half exchange (first half ran mid-loop)
                if no_collective:
                    for k in range(n_cores):
                        nc.gpsimd.dma_start(
                            out=cc_outB[k * Bh : (k + 1) * Bh, :],
                            in_=cc_in[Bh:Bc, :],
                        )
                else:
                    nc.gpsimd.collective_compute(
                        "AllGather",
                        mybir.AluOpType.bypass,
                        replica_groups=[list(range(n_cores))],
                        ins=[cc_in[Bh:Bc, :]],
                        outs=[cc_outB[:, :]],
                    )
                for k in range(n_cores):
                    nc.gpsimd.dma_start(
                        out=wag[k * Bc + Bh : (k + 1) * Bc, :],
                        in_=cc_outB[k * Bh : (k + 1) * Bh, :],
                    )

                # weighted half: transpose the gathered matrix on PE
                wTs = []
                for c in range(NT_W):
                    tp_ps = tpp.tile([P, B], tlo, tag="tp")
                    nc.tensor.transpose(
                        tp_ps, wag[:, c * P : (c + 1) * P], ident_sb[:B, :B]
                    )
                    wT = wTp.tile([P, B], tlo, tag="wT")
                    nc.vector.tensor_copy(wT, tp_ps)
                    wTs.append(wT)

                for c in range(NT_W):
                    nc.tensor.matmul(
                        out_ps,
                        wTs[c],
                        wt_lo[:, c, :],
                        start=False,
                        stop=(c == NT_W - 1),
                    )

                # bias add + store
                out_sb = tailp.tile([B, DOUT], f32)
                nc.vector.tensor_add(out_sb, out_ps, bias_bc)
                nc.sync.dma_start(out=out[:, :], in_=out_sb)

    nc.finalize()
    return nc


_CACHE = {}


def _get_program(S, B, D, n_cores):
    key = (S, B, D, n_cores)
    if key not in _CACHE:
        _CACHE[key] = build_program(S, B, D, n_cores)
    return _CACHE[key]


def make_in_maps(inp, hidden, W, b, n_cores=8):
    """Shard host inputs into per-core input maps."""
    S, B, D = inp.shape
    Bc = B // n_cores
    DOUT = W.shape[0] // n_cores
    P = 128
    F = 2 * D
    NKF = F // P
    NT = D // P
    lo = np.float16 if MODE == "fp16" else np.float32
    inp = np.asarray(inp, dtype=np.float32).astype(lo)
    hid0 = np.asarray(hidden[0], dtype=np.float32).astype(lo)   # [B, D]
    # hidden^T in partition-major chunk layout [P, NT, B]
    hidT = np.ascontiguousarray(hid0.T)                         # [D, B]
    hidT3 = np.ascontiguousarray(
        hidT.reshape(NT, P, B).transpose(1, 0, 2))
    wtT = np.asarray(W, dtype=np.float32).T.astype(lo)          # [F, 4096]
    ident = np.eye(128, dtype=np.float32).astype(lo)
    in_maps = []
    for k in range(n_cores):
        wtk = wtT[:, k * DOUT : (k + 1) * DOUT]                 # [F, DOUT]
        wt3 = np.ascontiguousarray(
            wtk.reshape(NKF, P, DOUT).transpose(1, 0, 2))       # [P, NKF, DOUT]
        in_maps.append(
            {
                "inp": np.ascontiguousarray(inp[:, k * Bc : (k + 1) * Bc, :]),
                "hid": np.ascontiguousarray(hid0[k * Bc : (k + 1) * Bc, :]),
                "hidT3": hidT3,
                "wt3": wt3,
                "bias": np.ascontiguousarray(
                    np.asarray(b[k * DOUT : (k + 1) * DOUT], dtype=np.float32)
                    .reshape(1, DOUT)
                ),
                "ident": ident,
            }
        )
    return in_maps


def kernel(inp, hidden, W, b, trace=False):
    from concourse.bass_utils import run_bass_kernel_spmd

    inp = np.asarray(inp, dtype=np.float32)
    hidden = np.asarray(hidden, dtype=np.float32)
    W = np.asarray(W, dtype=np.float32)
    b = np.asarray(b, dtype=np.float32)

    S, B, D = inp.shape
    n_cores = 8
    nc = _get_program(S, B, D, n_cores)
    in_maps = make_in_maps(inp, hidden, W, b, n_cores)
    res = run_bass_kernel_spmd(nc, in_maps, core_ids=list(range(n_cores)))
    outs = [res.results[k]["out"] for k in range(n_cores)]
    full = np.concatenate(outs, axis=1)  # [B, D]
    if trace:
        return full[None, :, :], res
    return full[None, :, :]
